# revision 7
# baseline (speedup 1.0000x reference)
"""Trainium2 Bass kernel for nn_Block_head (dense_transformer).

Problem: B=32, L=2048, D=512. 6 sequential TX blocks:
    scores = q @ k^T / sqrt(D)       (per-batch matvec)
    s      = softmax(scores)
    A      = s @ v
    q'     = norm(A + q)             (unbiased std, eps outside sqrt)
    ff     = relu(q' @ W1 + b1) @ W2 + b2
    q      = norm(q' + ff)
Outputs: out = concat(q after block 3, q after block 6) -> (32, 1024)
         weights = stack of s per block -> (32, 2048, 6)

Strategy: data-parallel over batch across 8 cores (4 rows each). k^T and v
live in SBUF as bf16 for the whole kernel; each block streams them through
the PE with 4-way column-tiled matmuls (one batch row per 32-column group,
concurrent via multiple XBUSes). Softmax skips the max-subtraction (scores
are O(1)); exp comes with its row-sum for free via ACT accum_out. The
normalization e/Z for the weights output happens on the host.
"""

import numpy as np
import ml_dtypes

B, L, D = 32, 2048, 512
DFF = D // 2
NBLK = 6
NCORES = 8
BLOC = B // NCORES          # 4 batch rows per core
EPS = 1e-6
CH_D = D // 128             # 4 contraction chunks for d
CH_DFF = DFF // 128         # 2 contraction chunks for d_ff
NLT = L // 512              # 4 l-tiles (PSUM width 512)
NLC = L // 128              # 16 l-chunks
BF16 = ml_dtypes.bfloat16

_cache = {}


def _build_program(use_ab1, use_ab2, use_b1, use_b2):
    import concourse.bacc as bacc
    import concourse.tile as tile
    from concourse import mybir
    from concourse.masks import make_identity

    f32 = mybir.dt.float32
    bf16 = mybir.dt.bfloat16
    Alu = mybir.AluOpType
    Act = mybir.ActivationFunctionType
    X = mybir.AxisListType.X

    nc = bacc.Bacc("TRN2", target_bir_lowering=False, debug=False)

    kt_d = nc.dram_tensor("kt", [BLOC, CH_D, 128, L], bf16, kind="ExternalInput")
    vv_d = nc.dram_tensor("vv", [BLOC, NLC, 128, D], bf16, kind="ExternalInput")
    w1_d = nc.dram_tensor("w1", [NBLK, CH_D, 128, DFF], bf16, kind="ExternalInput")
    w2_d = nc.dram_tensor("w2", [NBLK, CH_DFF, 128, D], bf16, kind="ExternalInput")
    q0_d = nc.dram_tensor("q0", [BLOC, D], f32, kind="ExternalInput")
    ab1_d = ab2_d = b1_d = b2_d = None
    if use_ab1:
        ab1_d = nc.dram_tensor("ab1", [2, NBLK, D], bf16, kind="ExternalInput")
    if use_ab2:
        ab2_d = nc.dram_tensor("ab2", [2, NBLK, D], bf16, kind="ExternalInput")
    if use_b1:
        b1_d = nc.dram_tensor("bb1", [NBLK, DFF], bf16, kind="ExternalInput")
    if use_b2:
        b2_d = nc.dram_tensor("bb2", [NBLK, D], bf16, kind="ExternalInput")

    e_out = nc.dram_tensor("e_out", [NBLK, BLOC, L], f32, kind="ExternalOutput")
    z_out = nc.dram_tensor("z_out", [NBLK, BLOC], f32, kind="ExternalOutput")
    q_out = nc.dram_tensor("q_out", [2, BLOC, D], f32, kind="ExternalOutput")

    P97 = 97  # covers partitions {0, 32, 64, 96}; rows in between are junk

    with tile.TileContext(nc) as tc:
        import contextlib

        with contextlib.ExitStack() as ctx:
            const = ctx.enter_context(tc.tile_pool(name="const", bufs=1))
            resid = ctx.enter_context(tc.tile_pool(name="resid", bufs=1))
            state = ctx.enter_context(tc.tile_pool(name="state", bufs=1))
            epool = ctx.enter_context(tc.tile_pool(name="epool", bufs=2))
            etp = ctx.enter_context(tc.tile_pool(name="etp", bufs=2))
            qtp = ctx.enter_context(tc.tile_pool(name="qtp", bufs=2))
            xp = ctx.enter_context(tc.tile_pool(name="xp", bufs=2))
            small = ctx.enter_context(tc.tile_pool(name="small", bufs=4))
            scp = ctx.enter_context(tc.tile_pool(name="scp", bufs=2, space="PSUM"))
            ap_ = ctx.enter_context(tc.tile_pool(name="ap", bufs=1, space="PSUM"))
            tpp = ctx.enter_context(tc.tile_pool(name="tpp", bufs=2, space="PSUM"))
            ffp = ctx.enter_context(tc.tile_pool(name="ffp", bufs=2, space="PSUM"))

            ident = const.tile([128, 128], f32)
            make_identity(nc, ident[:])
            # all-zero stationary operand: prefix matmul fills a PSUM tile's
            # unused partitions with 0.0 so later full-range engine reads see
            # deterministic data (and the simulator sees initialized memory)
            zero_sb = const.tile([128, 512], f32)
            nc.gpsimd.memset(zero_sb[:], 0.0)

            def psum_zero(ps_ap, width):
                # group-free PSUM fill so unused partitions read as 0.0
                nc.scalar.activation(
                    ps_ap, zero_sb[0:P97, 0:width], Act.Copy
                )

            # ---- resident loads -------------------------------------------------
            w1_sb = resid.tile([128, NBLK, CH_D, DFF], bf16, tag="w1")
            nc.sync.dma_start(w1_sb[:], w1_d.ap().rearrange("i c p f -> p i c f"))
            w2_sb = resid.tile([128, NBLK, CH_DFF, D], bf16, tag="w2")
            nc.sync.dma_start(w2_sb[:], w2_d.ap().rearrange("i c p f -> p i c f"))

            # interleave k-tile and v-chunk loads in block-1 consumption order
            # so the first block streams: scores(t) needs kt[t]; the A-matmuls
            # for chunks 4t..4t+3 need v[4t..4t+3] right after.
            kt_sb = [None] * NLT
            v_sb = [None] * NLC
            for t in range(NLT):
                kt_t = resid.tile([128, BLOC, CH_D, 512], bf16, tag=f"kt{t}")
                nc.sync.dma_start(
                    kt_t[:],
                    kt_d.ap()[:, :, :, t * 512 : (t + 1) * 512].rearrange(
                        "b c p l -> p b c l"
                    ),
                )
                kt_sb[t] = kt_t
                for lc in range(4 * t, 4 * t + 4):
                    v_t = resid.tile([128, BLOC, D], bf16, tag=f"v{lc}")
                    nc.sync.dma_start(
                        v_t[:], vv_d.ap()[:, lc].rearrange("b p d -> p b d")
                    )
                    v_sb[lc] = v_t

            q_sb = state.tile([128, D], f32, tag="q")
            nc.gpsimd.memset(q_sb[:], 0.0)
            for b in range(BLOC):
                nc.sync.dma_start(
                    q_sb[32 * b : 32 * b + 1, :], q0_d.ap()[b : b + 1, :]
                )

            def load_rep(dram_ap, n):
                t = resid.tile([128, n], bf16, tag=f"rep{id(dram_ap)}")
                for b in range(BLOC):
                    nc.sync.dma_start(t[32 * b : 32 * b + 1, :], dram_ap)
                return t

            a1_sb = b1n_sb = a2_sb = b2n_sb = fb1_sb = fb2_sb = None
            if use_ab1:
                a1_sb = load_rep(ab1_d.ap()[0].rearrange("i d -> (i d)")[None, :], NBLK * D)
                b1n_sb = load_rep(ab1_d.ap()[1].rearrange("i d -> (i d)")[None, :], NBLK * D)
            if use_ab2:
                a2_sb = load_rep(ab2_d.ap()[0].rearrange("i d -> (i d)")[None, :], NBLK * D)
                b2n_sb = load_rep(ab2_d.ap()[1].rearrange("i d -> (i d)")[None, :], NBLK * D)
            if use_b1:
                fb1_sb = load_rep(b1_d.ap().rearrange("i f -> (i f)")[None, :], NBLK * DFF)
            if use_b2:
                fb2_sb = load_rep(b2_d.ap().rearrange("i f -> (i f)")[None, :], NBLK * D)

            zfin = state.tile([128, NBLK], f32, tag="zfin")

            # ---- helpers --------------------------------------------------------
            def transpose_state(src_sb, src_off, nch, dst_dtype=bf16):
                """src rows {0,32,64,96}, cols [src_off, src_off+nch*128) (f32)
                -> [128, nch, 4] tile (dst_dtype), dst[p, c, b] = src[32b, src_off+128c+p]."""
                dst = qtp.tile([128, nch, BLOC], dst_dtype, tag=f"qt{nch}")
                for c in range(nch):
                    tp = tpp.tile([128, P97], f32, tag="tp")
                    nc.tensor.transpose(
                        tp[:],
                        src_sb[0:P97, src_off + c * 128 : src_off + (c + 1) * 128],
                        ident[0:P97, 0:P97],
                    )
                    nc.vector.tensor_copy(dst[:, c, :], tp[:, 0:P97:32])
                return dst

            def norm(src_sb, dst_sb, alpha_rep, bias_rep, idx):
                """dst = alpha*(src-mean)/(std+eps)+bias along free dim (D), rows {0,32,64,96}."""
                st6 = small.tile([128, 6], f32, tag="st6")
                nc.vector.bn_stats(st6[0:P97, :], src_sb[0:P97, :])
                st2 = small.tile([128, 2], f32, tag="st2")
                nc.vector.bn_aggr(st2[0:P97, :], st6[0:P97, :])
                std = small.tile([128, 1], f32, tag="std")
                nc.scalar.activation(
                    std[0:P97, :], st2[0:P97, 1:2], Act.Sqrt, scale=float(D) / (D - 1)
                )
                den = small.tile([128, 1], f32, tag="den")
                nc.vector.tensor_scalar_add(den[0:P97, :], std[0:P97, :], EPS)
                inv = small.tile([128, 1], f32, tag="inv")
                nc.vector.reciprocal(inv[0:P97, :], den[0:P97, :])
                nc.vector.tensor_scalar(
                    dst_sb[0:P97, :],
                    src_sb[0:P97, :],
                    st2[0:P97, 0:1],
                    inv[0:P97, 0:1],
                    op0=Alu.subtract,
                    op1=Alu.mult,
                )
                if alpha_rep is not None:
                    nc.vector.scalar_tensor_tensor(
                        dst_sb[0:P97, :],
                        dst_sb[0:P97, :],
                        1.0,
                        alpha_rep[0:P97, idx * D : (idx + 1) * D],
                        op0=Alu.mult,
                        op1=Alu.mult,
                    )
                if bias_rep is not None:
                    nc.vector.scalar_tensor_tensor(
                        dst_sb[0:P97, :],
                        dst_sb[0:P97, :],
                        0.0,
                        bias_rep[0:P97, idx * D : (idx + 1) * D],
                        op0=Alu.add,
                        op1=Alu.add,
                    )

            qT = transpose_state(q_sb, 0, CH_D)
            inv_sqrt_d = float(1.0 / np.sqrt(D))

            # ---- the 6 sequential TX blocks ------------------------------------
            for idx in range(NBLK):
                a_ps = ap_.tile([128, D], f32, tag="a")
                psum_zero(a_ps[0:P97, :], 512)
                e_sb = epool.tile([128, L], f32, tag="e")
                eT = etp.tile([128, NLC, BLOC], bf16, tag="eT")
                zp = small.tile([128, NLT], f32, tag="zp")

                for t in range(NLT):
                    sc = scp.tile([128, 512], f32, tag="sc")
                    psum_zero(sc[0:P97, :], 512)
                    for c in range(CH_D):
                        for b in range(BLOC):
                            nc.tensor.matmul(
                                sc[32 * b : 32 * b + 1, :],
                                lhsT=qT[:, c, b : b + 1],
                                rhs=kt_sb[t][:, b, c, :],
                                start=(c == 0),
                                stop=(c == CH_D - 1),
                                tile_position=(0, 32 * b),
                            )
                    # e = exp(scores / sqrt(D)); row-sums accumulate into zp
                    nc.scalar.activation(
                        e_sb[0:P97, t * 512 : (t + 1) * 512],
                        sc[0:P97, :],
                        Act.Exp,
                        scale=inv_sqrt_d,
                        accum_out=zp[0:P97, t : t + 1],
                    )
                    for j in range(4):
                        lc = t * 4 + j
                        tp = tpp.tile([128, P97], f32, tag="tp")
                        nc.tensor.transpose(
                            tp[:],
                            e_sb[0:P97, lc * 128 : (lc + 1) * 128],
                            ident[0:P97, 0:P97],
                        )
                        nc.vector.tensor_copy(eT[:, lc, :], tp[:, 0:P97:32])
                        for b in range(BLOC):
                            nc.tensor.matmul(
                                a_ps[32 * b : 32 * b + 1, :],
                                lhsT=eT[:, lc, b : b + 1],
                                rhs=v_sb[lc][:, b, :],
                                start=(lc == 0),
                                stop=(lc == NLC - 1),
                                tile_position=(0, 32 * b),
                            )

                nc.vector.reduce_sum(zfin[0:P97, idx : idx + 1], zp[0:P97, :], axis=X)
                invz = small.tile([128, 1], f32, tag="invz")
                nc.vector.reciprocal(invz[0:P97, :], zfin[0:P97, idx : idx + 1])
                for b in range(BLOC):
                    nc.sync.dma_start(
                        e_out.ap()[idx, b, :], e_sb[32 * b : 32 * b + 1, :]
                    )

                # x = A/Z + q
                x_sb = xp.tile([128, D], f32, tag="x")
                nc.vector.scalar_tensor_tensor(
                    x_sb[0:P97, :],
                    a_ps[0:P97, :],
                    invz[0:P97, 0:1],
                    q_sb[0:P97, :],
                    op0=Alu.mult,
                    op1=Alu.add,
                )
                qm_sb = xp.tile([128, D], f32, tag="qm")
                norm(x_sb, qm_sb, a1_sb, b1n_sb, idx)
                qmT = transpose_state(qm_sb, 0, CH_D)

                # FF1: h = relu(qm @ W1 [+ b1])
                h_ps = ffp.tile([128, D], f32, tag="ff")
                psum_zero(h_ps[0:P97, 0:DFF], DFF)
                for c in range(CH_D):
                    for b in range(BLOC):
                        nc.tensor.matmul(
                            h_ps[32 * b : 32 * b + 1, 0:DFF],
                            lhsT=qmT[:, c, b : b + 1],
                            rhs=w1_sb[:, idx, c, :],
                            start=(c == 0),
                            stop=(c == CH_D - 1),
                            tile_position=(0, 32 * b),
                        )
                h_sb = xp.tile([128, DFF], f32, tag="h")
                if use_b1:
                    nc.vector.scalar_tensor_tensor(
                        h_sb[0:P97, :],
                        h_ps[0:P97, 0:DFF],
                        0.0,
                        fb1_sb[0:P97, idx * DFF : (idx + 1) * DFF],
                        op0=Alu.add,
                        op1=Alu.add,
                    )
                    nc.scalar.activation(h_sb[0:P97, :], h_sb[0:P97, :], Act.Relu)
                else:
                    nc.scalar.activation(h_sb[0:P97, :], h_ps[0:P97, 0:DFF], Act.Relu)
                hT = transpose_state(h_sb, 0, CH_DFF)

                # FF2: ff = h @ W2 [+ b2]
                f_ps = ffp.tile([128, D], f32, tag="ff")
                psum_zero(f_ps[0:P97, :], 512)
                for c in range(CH_DFF):
                    for b in range(BLOC):
                        nc.tensor.matmul(
                            f_ps[32 * b : 32 * b + 1, :],
                            lhsT=hT[:, c, b : b + 1],
                            rhs=w2_sb[:, idx, c, :],
                            start=(c == 0),
                            stop=(c == CH_DFF - 1),
                            tile_position=(0, 32 * b),
                        )
                # x2 = qm + ff [+ b2]
                x2_sb = xp.tile([128, D], f32, tag="x")
                nc.vector.scalar_tensor_tensor(
                    x2_sb[0:P97, :],
                    f_ps[0:P97, :],
                    0.0,
                    qm_sb[0:P97, :],
                    op0=Alu.add,
                    op1=Alu.add,
                )
                if use_b2:
                    nc.vector.scalar_tensor_tensor(
                        x2_sb[0:P97, :],
                        x2_sb[0:P97, :],
                        0.0,
                        fb2_sb[0:P97, idx * D : (idx + 1) * D],
                        op0=Alu.add,
                        op1=Alu.add,
                    )
                norm(x2_sb, q_sb, a2_sb, b2n_sb, idx)
                if idx in (2, NBLK - 1):
                    for b in range(BLOC):
                        nc.sync.dma_start(
                            q_out.ap()[idx // 3, b, :], q_sb[32 * b : 32 * b + 1, :]
                        )
                if idx < NBLK - 1:
                    qT = transpose_state(q_sb, 0, CH_D)

            for b in range(BLOC):
                nc.sync.dma_start(z_out.ap()[:, b], zfin[32 * b : 32 * b + 1, :])

    nc.compile()
    return nc


def _get_program(use_ab1, use_ab2, use_b1, use_b2):
    key = (use_ab1, use_ab2, use_b1, use_b2)
    if key not in _cache:
        _cache[key] = _build_program(*key)
    return _cache[key]


def _make_in_maps(q, k, v, W1, b1, W2, b2, alpha1, bias1, alpha2, bias2,
                  use_ab1, use_ab2, use_b1, use_b2):
    w1_np = np.ascontiguousarray(W1.reshape(NBLK, CH_D, 128, DFF)).astype(BF16)
    w2_np = np.ascontiguousarray(W2.reshape(NBLK, CH_DFF, 128, D)).astype(BF16)
    shared = {"w1": w1_np, "w2": w2_np}
    if use_ab1:
        shared["ab1"] = np.stack([alpha1, bias1]).astype(BF16)
    if use_ab2:
        shared["ab2"] = np.stack([alpha2, bias2]).astype(BF16)
    if use_b1:
        shared["bb1"] = b1.astype(BF16)
    if use_b2:
        shared["bb2"] = b2.astype(BF16)

    in_maps = []
    for ci in range(NCORES):
        sl = slice(ci * BLOC, (ci + 1) * BLOC)
        k_loc = k[sl]  # (BLOC, L, D)
        kt_np = np.ascontiguousarray(
            k_loc.transpose(0, 2, 1).reshape(BLOC, CH_D, 128, L)
        ).astype(BF16)
        vv_np = np.ascontiguousarray(
            v[sl].reshape(BLOC, NLC, 128, D)
        ).astype(BF16)
        q_np = np.ascontiguousarray(q[sl]).astype(np.float32)
        in_maps.append({"kt": kt_np, "vv": vv_np, "q0": q_np, **shared})
    return in_maps


def _assemble(results):
    out = np.empty((B, 2 * D), dtype=np.float32)
    weights = np.empty((B, L, NBLK), dtype=np.float32)
    for ci, res in enumerate(results):
        sl = slice(ci * BLOC, (ci + 1) * BLOC)
        qo = res["q_out"]          # (2, BLOC, D)
        out[sl, 0:D] = qo[0]
        out[sl, D : 2 * D] = qo[1]
        e = res["e_out"]           # (NBLK, BLOC, L)
        z = res["z_out"]           # (NBLK, BLOC)
        s = e / z[:, :, None]      # fp32 softmax normalization
        weights[sl] = s.transpose(1, 2, 0)
    return out, weights


def kernel(q, k, v, W1, b1, W2, b2, alpha1, bias1, alpha2, bias2):
    q = np.asarray(q, np.float32)
    k = np.asarray(k, np.float32)
    v = np.asarray(v, np.float32)
    W1 = np.asarray(W1, np.float32)
    W2 = np.asarray(W2, np.float32)
    b1 = np.asarray(b1, np.float32)
    b2 = np.asarray(b2, np.float32)
    alpha1 = np.asarray(alpha1, np.float32)
    bias1 = np.asarray(bias1, np.float32)
    alpha2 = np.asarray(alpha2, np.float32)
    bias2 = np.asarray(bias2, np.float32)

    use_ab1 = not (np.all(alpha1 == 1.0) and np.all(bias1 == 0.0))
    use_ab2 = not (np.all(alpha2 == 1.0) and np.all(bias2 == 0.0))
    use_b1 = not np.all(b1 == 0.0)
    use_b2 = not np.all(b2 == 0.0)

    from concourse.bass_utils import run_bass_kernel_spmd

    nc = _get_program(use_ab1, use_ab2, use_b1, use_b2)
    in_maps = _make_in_maps(q, k, v, W1, b1, W2, b2, alpha1, bias1,
                            alpha2, bias2, use_ab1, use_ab2, use_b1, use_b2)
    br = run_bass_kernel_spmd(nc, in_maps, list(range(NCORES)))
    return _assemble(br.results)
